# revision 5
# baseline (speedup 1.0000x reference)
"""HaarDeconv2D (vertical, 2x1, stride (2,1)) Trainium2 kernel.

Math: with L=[0.5,0.5], D=[0.5,-0.5],
  even = 0.5*(low+detail) + 0.5*(low-detail) = low_pass
  odd  = 0.5*(low+detail) - 0.5*(low-detail) = detail
so the output is exactly a row-interleave of the two inputs along H:
pure data movement, done as strided DRAM->DRAM DMA (contiguous write
stream, two sequential read cursors), no compute engines.

Load balancing: of the 8 tunneled NeuronCores, cores 2 and 3 share an
HBM stack (measured ~186 GB/s each vs ~650 GB/s for the other six), so
an even shard makes them the straggler. The global row-pair space
(B*C*H = 24576 rows) is split unevenly: work is issued in 64-row-pair
chunks, every core runs the same NEFF with K_MAX predicated chunk DMAs,
and a per-core int32 input `nck` selects how many chunks are real
(cond=False DMAs are skipped via the OOB mechanism but still increment
the completion semaphore).
"""

import numpy as np

_N_CORES = 8
_B, _C, _H, _W = 16, 3, 512, 1024
_RTOT = _B * _C * _H  # 24576 global row pairs
_RCHUNK = 64  # row pairs per chunk DMA
_KMAX = 55  # max chunks per core
_NMAX = _KMAX * _RCHUNK  # 3520 row pairs per core max

# chunks per core: sums to RTOT/RCHUNK = 384. Cores 2,3 share an HBM
# stack -> about half the bandwidth of the others.
_COUNTS = [55, 55, 27, 27, 55, 55, 55, 55]
assert sum(_COUNTS) == _RTOT // _RCHUNK
assert max(_COUNTS) <= _KMAX

_nc_cache = None


def _build():
    global _nc_cache
    if _nc_cache is not None:
        return _nc_cache
    import concourse.bass as bass
    import concourse.bacc as bacc
    import concourse.mybir as mybir

    nc = bacc.Bacc()
    inp = nc.dram_tensor(
        "inp", [2, _NMAX, _W], mybir.dt.float32, kind="ExternalInput"
    )
    nck = nc.dram_tensor("nck", [1, 1], mybir.dt.int32, kind="ExternalInput")
    out = nc.dram_tensor(
        "out", [_NMAX, 2 * _W], mybir.dt.float32, kind="ExternalOutput"
    )

    with (
        nc.Block() as block,
        nc.semaphore("dma_sem") as dma_sem,
        nc.sync.register() as nck_reg,
    ):

        @block.sync
        def _(sync):
            sync.reg_load(nck_reg, nck[0:1, 0:1])
            n = sync.snap(nck_reg, min_val=0, max_val=_KMAX)
            for k in range(_KMAX):
                src_k = inp[:, k * _RCHUNK : (k + 1) * _RCHUNK, :].rearrange(
                    "s m w -> m s w"
                )
                dst_k = out[k * _RCHUNK : (k + 1) * _RCHUNK, :]
                sync.dma_start(out=dst_k, in_=src_k, cond=(k < n)).then_inc(
                    dma_sem, 16
                )
            sync.wait_ge(dma_sem, 16 * _KMAX)

    nc.compile()
    _nc_cache = nc
    return nc


def _shard_inputs(low_pass, detail):
    low_pass = np.asarray(low_pass, dtype=np.float32)
    detail = np.asarray(detail, dtype=np.float32)
    lo = low_pass.reshape(_RTOT, _W)
    de = detail.reshape(_RTOT, _W)
    in_maps = []
    o = 0
    for i in range(_N_CORES):
        n = _COUNTS[i] * _RCHUNK
        buf = np.zeros((2, _NMAX, _W), dtype=np.float32)
        buf[0, :n] = lo[o : o + n]
        buf[1, :n] = de[o : o + n]
        in_maps.append(
            {"inp": buf, "nck": np.array([[_COUNTS[i]]], dtype=np.int32)}
        )
        o += n
    return in_maps


def _gather_outputs(results):
    parts = []
    for i in range(_N_CORES):
        n = _COUNTS[i] * _RCHUNK
        parts.append(results[i]["out"][:n])
    full = np.concatenate(parts, axis=0)  # [RTOT, 2W]
    return full.reshape(_B, _C, 2 * _H, _W)


def kernel(low_pass, detail):
    from concourse.bass_utils import run_bass_kernel_spmd

    nc = _build()
    in_maps = _shard_inputs(low_pass, detail)
    r = run_bass_kernel_spmd(nc, in_maps, core_ids=list(range(_N_CORES)))
    return _gather_outputs(r.results)


# revision 10
# speedup vs baseline: 1.0980x; 1.0980x over previous
"""HaarDeconv2D (vertical, 2x1, stride (2,1)) Trainium2 kernel.

Math: with L=[0.5,0.5], D=[0.5,-0.5],
  even = 0.5*(low+detail) + 0.5*(low-detail) = low_pass
  odd  = 0.5*(low+detail) - 0.5*(low-detail) = detail
so the output is exactly a row-interleave of the two inputs along H:
pure data movement, done as strided DRAM->DRAM DMA (contiguous write
stream, two sequential read cursors), no compute engines involved.
The host packs each core's (low, detail) shard into one stacked input
(pure concatenation); the interleave itself happens on device.

Load balancing: per-core HBM bandwidth differs between the 8 tunneled
NeuronCores (measured stable classes: cores {0,4,6} ~0.26 MB/us of
output bytes, the rest ~0.29). The global row-pair space
(B*C*H = 24576 rows) is split unevenly in RCHUNK-row chunks: every
core runs the same SPMD NEFF with KMAX predicated chunk DMAs and a
per-core int32 input `nck` selects how many chunks are real
(cond=False DMAs are skipped via the OOB mechanism but still increment
the completion semaphore), so the split is host-tunable without
recompiling.
"""

import numpy as np

_N_CORES = 8
_B, _C, _H, _W = 16, 3, 512, 1024
_RTOT = _B * _C * _H  # 24576 global row pairs

_RCHUNK = 128  # row pairs per chunk DMA (1 MiB of output)
_KMAX = 30  # max chunks per core
_NMAX = _KMAX * _RCHUNK  # row pairs per core max

# chunks per core; sums to RTOT/RCHUNK = 192. Cores {0,4,6} are the
# measured slower class and get less work.
_COUNTS = [22, 25, 25, 25, 23, 25, 22, 25]
assert sum(_COUNTS) == _RTOT // _RCHUNK
assert max(_COUNTS) <= _KMAX

_nc_cache = None


def _build():
    global _nc_cache
    if _nc_cache is not None:
        return _nc_cache
    import concourse.bacc as bacc
    import concourse.mybir as mybir

    nc = bacc.Bacc()
    inp = nc.dram_tensor(
        "inp", [2, _NMAX, _W], mybir.dt.float32, kind="ExternalInput"
    )
    nck = nc.dram_tensor("nck", [1, 1], mybir.dt.int32, kind="ExternalInput")
    out = nc.dram_tensor(
        "out", [_NMAX, 2 * _W], mybir.dt.float32, kind="ExternalOutput"
    )

    with (
        nc.Block() as block,
        nc.semaphore("dma_sem") as dma_sem,
        nc.sync.register() as nck_reg,
    ):

        @block.sync
        def _(sync):
            sync.reg_load(nck_reg, nck[0:1, 0:1])
            n = sync.snap(nck_reg, min_val=0, max_val=_KMAX)
            for k in range(_KMAX):
                # src read order (m, s, w) makes the write stream of
                # dst fully contiguous
                src_k = inp[:, k * _RCHUNK : (k + 1) * _RCHUNK, :].rearrange(
                    "s m w -> m s w"
                )
                dst_k = out[k * _RCHUNK : (k + 1) * _RCHUNK, :]
                sync.dma_start(out=dst_k, in_=src_k, cond=(k < n)).then_inc(
                    dma_sem, 16
                )
            sync.wait_ge(dma_sem, 16 * _KMAX)

    nc.compile()
    _nc_cache = nc
    return nc


def _shard_inputs(low_pass, detail):
    low_pass = np.asarray(low_pass, dtype=np.float32)
    detail = np.asarray(detail, dtype=np.float32)
    lo = low_pass.reshape(_RTOT, _W)
    de = detail.reshape(_RTOT, _W)
    in_maps = []
    o = 0
    for i in range(_N_CORES):
        n = _COUNTS[i] * _RCHUNK
        buf = np.zeros((2, _NMAX, _W), dtype=np.float32)
        buf[0, :n] = lo[o : o + n]
        buf[1, :n] = de[o : o + n]
        in_maps.append(
            {"inp": buf, "nck": np.array([[_COUNTS[i]]], dtype=np.int32)}
        )
        o += n
    return in_maps


def _gather_outputs(results):
    parts = []
    for i in range(_N_CORES):
        n = _COUNTS[i] * _RCHUNK
        parts.append(results[i]["out"][:n])
    full = np.concatenate(parts, axis=0)  # [RTOT, 2W]
    return full.reshape(_B, _C, 2 * _H, _W)


def kernel(low_pass, detail):
    from concourse.bass_utils import run_bass_kernel_spmd

    nc = _build()
    in_maps = _shard_inputs(low_pass, detail)
    r = run_bass_kernel_spmd(nc, in_maps, core_ids=list(range(_N_CORES)))
    return _gather_outputs(r.results)


# revision 11
# speedup vs baseline: 1.2725x; 1.1589x over previous
"""HaarDeconv2D (vertical, 2x1, stride (2,1)) Trainium2 kernel.

Math: with L=[0.5,0.5], D=[0.5,-0.5],
  even = 0.5*(low+detail) + 0.5*(low-detail) = low_pass
  odd  = 0.5*(low+detail) - 0.5*(low-detail) = detail
so the output is exactly a row-interleave of the two inputs along H:
pure data movement, done as strided DRAM->DRAM DMA (contiguous write
stream, two sequential read cursors), no compute engines involved.
The host packs each core's (low, detail) shard into one stacked input
(pure concatenation); the interleave itself happens on device.

Load balancing: per-core HBM bandwidth differs between the 8 tunneled
NeuronCores (measured stable classes: cores {0,4,6} ~0.26 MB/us of
output bytes, the rest ~0.29). The global row-pair space
(B*C*H = 24576 rows) is split unevenly in RCHUNK-row chunks: every
core runs the same SPMD NEFF with KMAX predicated chunk DMAs and a
per-core int32 input `nck` selects how many chunks are real
(cond=False DMAs are skipped via the OOB mechanism but still increment
the completion semaphore), so the split is host-tunable without
recompiling.
"""

import numpy as np

_N_CORES = 8
_B, _C, _H, _W = 16, 3, 512, 1024
_RTOT = _B * _C * _H  # 24576 global row pairs

_RCHUNK = 128  # row pairs per chunk DMA (1 MiB of output)
_KMAX = 30  # max chunks per core
_NMAX = _KMAX * _RCHUNK  # row pairs per core max

# chunks per core; sums to RTOT/RCHUNK = 192. Cores {0,4,6} are the
# measured slower class and get less work.
_COUNTS = [22, 25, 25, 25, 23, 25, 22, 25]
assert sum(_COUNTS) == _RTOT // _RCHUNK
assert max(_COUNTS) <= _KMAX

_nc_cache = None


def _build():
    global _nc_cache
    if _nc_cache is not None:
        return _nc_cache
    import concourse.bacc as bacc
    import concourse.mybir as mybir

    nc = bacc.Bacc()
    inp = nc.dram_tensor(
        "inp", [2, _NMAX, _W], mybir.dt.float32, kind="ExternalInput"
    )
    nck = nc.dram_tensor("nck", [1, 1], mybir.dt.int32, kind="ExternalInput")
    out = nc.dram_tensor(
        "out", [_NMAX, 2 * _W], mybir.dt.float32, kind="ExternalOutput"
    )

    with (
        nc.Block() as block,
        nc.semaphore("dma_sem") as dma_sem,
        nc.sync.register() as nck_reg,
    ):

        kmin = min(_COUNTS)  # chunks below kmin are valid on every core

        @block.sync
        def _(sync):
            def chunk_aps(k):
                # src read order (m, s, w) makes the write stream of
                # dst fully contiguous
                src_k = inp[:, k * _RCHUNK : (k + 1) * _RCHUNK, :].rearrange(
                    "s m w -> m s w"
                )
                dst_k = out[k * _RCHUNK : (k + 1) * _RCHUNK, :]
                return src_k, dst_k

            # unconditional chunks first: no dependency on the nck load,
            # so the first DMA issues immediately
            for k in range(kmin):
                src_k, dst_k = chunk_aps(k)
                sync.dma_start(out=dst_k, in_=src_k).then_inc(dma_sem, 16)
            # nck load overlaps with the in-flight DMAs
            sync.reg_load(nck_reg, nck[0:1, 0:1])
            n = sync.snap(nck_reg, min_val=0, max_val=_KMAX)
            for k in range(kmin, _KMAX):
                src_k, dst_k = chunk_aps(k)
                sync.dma_start(out=dst_k, in_=src_k, cond=(k < n)).then_inc(
                    dma_sem, 16
                )
            sync.wait_ge(dma_sem, 16 * _KMAX)

    nc.compile()
    _nc_cache = nc
    return nc


def _shard_inputs(low_pass, detail):
    low_pass = np.asarray(low_pass, dtype=np.float32)
    detail = np.asarray(detail, dtype=np.float32)
    lo = low_pass.reshape(_RTOT, _W)
    de = detail.reshape(_RTOT, _W)
    in_maps = []
    o = 0
    for i in range(_N_CORES):
        n = _COUNTS[i] * _RCHUNK
        buf = np.zeros((2, _NMAX, _W), dtype=np.float32)
        buf[0, :n] = lo[o : o + n]
        buf[1, :n] = de[o : o + n]
        in_maps.append(
            {"inp": buf, "nck": np.array([[_COUNTS[i]]], dtype=np.int32)}
        )
        o += n
    return in_maps


def _gather_outputs(results):
    parts = []
    for i in range(_N_CORES):
        n = _COUNTS[i] * _RCHUNK
        parts.append(results[i]["out"][:n])
    full = np.concatenate(parts, axis=0)  # [RTOT, 2W]
    return full.reshape(_B, _C, 2 * _H, _W)


def kernel(low_pass, detail):
    from concourse.bass_utils import run_bass_kernel_spmd

    nc = _build()
    in_maps = _shard_inputs(low_pass, detail)
    r = run_bass_kernel_spmd(nc, in_maps, core_ids=list(range(_N_CORES)))
    return _gather_outputs(r.results)
